# revision 40
# baseline (speedup 1.0000x reference)
"""Causal self-attention (B=4, T=2048, HID=768, H=12) on 8 NeuronCores.

Sharding: core c handles batch b=c//2 and head-half c%2 (6 of 12 heads).
Data-parallel on B, tensor-parallel on heads; no cross-device communication.

Per-core kernel:
  - all matmul operands bf16 (PSUM accumulation fp32); P tiles for
    off-diagonal key chunks are fp8e4 and feed DoubleRow ctx matmuls
    (two 128-key chunks contracted per pass, 2x PE throughput there).
    Diagonal chunks stay bf16, so the first 128 queries (tiny softmax
    support, sensitive to value quantization) never touch fp8.
  - host pre-packs x and W so every DMA moves 4-6KB contiguous lines per
    partition (128 descriptors per DMA); DMAs are spread across the
    sync/scalar/gpsimd DGE queues so issue time doesn't serialize.
  - qT/kT laid out [128=2 heads x 64d, token] per pair; scores are
    computed transposed S^T[k, q] per 128-key chunk into [128, 2, 512]
    PSUM; one ACT exp per chunk covers both heads (scale=1/8, additive
    mask as per-partition bias).  Causal masking = column-range
    restriction + gpsimd affine_select on the exp'd diagonal block.
  - V is augmented with a 65th all-ones column so the ctx matmul
    accumulates ctx_num^T = P V and the softmax denominator Z in one
    [65, 512] PSUM tile; normalization = reciprocal_approx_fast ->
    gpsimd partition_broadcast -> DVE multiply -> bf16 output.
  - work units (head-pair, 512-query chunk) run j=0..3 so the x stream
    arrives just ahead of use; filler matmuls keep the PE HAM warm
    through the initial DMA phase.
"""

import sys
from collections import deque

for _p in ("/root/.axon_site/_ro/trn_rl_repo", "/opt/trn_rl_repo"):
    if _p not in sys.path:
        sys.path.append(_p)

import ml_dtypes
import numpy as np

import concourse.bass as bass
import concourse.mybir as mybir
import concourse.tile as tile
from concourse import bacc
from concourse.bass_utils import run_bass_kernel_spmd

F32 = mybir.dt.float32
BF16 = mybir.dt.bfloat16
F8 = mybir.dt.float8e4
DR = mybir.MatmulPerfMode.DoubleRow

B, T, HID, H = 4, 2048, 768, 12
D = HID // H            # 64
NH = 6                  # heads per core
NPAIR = 3               # head pairs per core
OC = NH * D             # 384 output dims per core
NCI = HID // 128        # 6 contraction chunks
NJ = T // 512           # 4 query chunks of 512
NT16 = T // 128         # 16 token chunks of 128

_TRACE = False
_TMPDIR = None
LAST_EXEC_NS = None
_COMPILED = None


def _install_trace_hook():
    import types

    if "antenv.axon_hooks" in sys.modules:
        return
    mod = types.ModuleType("antenv.axon_hooks")
    mod._hook = None
    mod.set_axon_ntff_profile_hook = lambda h: setattr(mod, "_hook", h)
    mod.get_axon_ntff_profile_hook = lambda: mod._hook
    sys.modules["antenv.axon_hooks"] = mod
    sys.path.insert(0, "/root/.axon_site")
    from trn_agent_boot.trn_boot import _ntff_profile_via_ctypes

    mod.set_axon_ntff_profile_hook(
        _ntff_profile_via_ctypes("/opt/axon/libaxon_pjrt.so")
    )


class _Unit:
    """One (head-pair, q-chunk-of-512) attention work unit."""

    def __init__(self, pi, j, slot):
        self.pi = pi
        self.j = j
        self.slot = slot
        self.nk = 4 * (j + 1)
        self.kc = 0
        self.sq = deque()    # scores awaiting exp (1-step delay)
        self.pend = deque()  # exp'd items awaiting ctx (1-step delay)
        self.cur8 = None     # fp8 pt tile collecting the current kc pair
        self.ctx = None


def _build():
    nc = bacc.Bacc("TRN2", target_bir_lowering=False)

    # host pre-packed layouts: per-partition contiguous lines
    xT = nc.dram_tensor("xT", [128, NJ * NCI * 512], BF16, kind="ExternalInput")
    wqT = nc.dram_tensor("wqT", [128, NCI * OC], BF16, kind="ExternalInput")
    wkT = nc.dram_tensor("wkT", [128, NCI * OC], BF16, kind="ExternalInput")
    wvT = nc.dram_tensor("wvT", [128, NCI * OC], BF16, kind="ExternalInput")
    bqT = nc.dram_tensor("bqT", [128, NPAIR], F32, kind="ExternalInput")
    bkT = nc.dram_tensor("bkT", [128, NPAIR], F32, kind="ExternalInput")
    bv = nc.dram_tensor("bv", [OC], F32, kind="ExternalInput")
    maskT = nc.dram_tensor("maskT", [128, NT16], F32, kind="ExternalInput")
    outT = nc.dram_tensor("outT", [OC, T], BF16, kind="ExternalOutput")

    with tile.TileContext(nc) as tc:
        consts = tc.alloc_tile_pool(name="consts", bufs=1)
        qk_pool = tc.alloc_tile_pool(name="qk", bufs=1)
        va_pool = tc.alloc_tile_pool(name="va", bufs=1)

        # ---- constants ----
        bq_t = consts.tile([128, NPAIR], F32, tag="bq")
        bk_t = consts.tile([128, NPAIR], F32, tag="bk")
        bvr = consts.tile([128, NH, D], F32, tag="bvr")
        mk_t = consts.tile([128, NT16], F32, tag="mk")


        # persistent activations
        qT = qk_pool.tile([128, NPAIR, T], BF16, tag="qT")
        kT = qk_pool.tile([128, NPAIR, T], BF16, tag="kT")
        va16 = va_pool.tile([128, NT16, NH, D + 1], BF16, tag="va16")
        # padded to 72B per head so the DoubleRow pair stride (NH*72) is
        # 16B-aligned as checkMatmultPerfMode requires
        va8 = va_pool.tile([128, NT16, NH, 72], F8, tag="va8")
        ones = consts.tile([128, 1], F32, tag="ones", name="ones")
        nc.vector.memset(ones, 1.0)

        pin_p = tc.alloc_tile_pool(name="pin", bufs=1)
        xt = pin_p.tile([128, NJ, NCI, 512], BF16, tag="xt")
        wq_t = pin_p.tile([128, NCI, OC], BF16, tag="wq")
        wk_t = pin_p.tile([128, NCI, OC], BF16, tag="wk")
        wv_t = pin_p.tile([128, NCI, OC], BF16, tag="wv")

        # batched input loads: x chunks on sync, weights on scalar/gpsimd;
        # small tensors issue after the big ones so descriptor generation
        # for the critical weights starts immediately.
        nc.sync.dma_start(out=xt[:, 0], in_=xT[:, 0:3072])
        nc.scalar.dma_start(out=wq_t, in_=wqT[:, :])
        nc.gpsimd.dma_start(out=mk_t, in_=maskT[:, :])
        nc.gpsimd.dma_start(out=wv_t, in_=wvT[:, :])
        nc.scalar.dma_start(out=wk_t, in_=wkT[:, :])
        for tj in (1, 2, 3):
            nc.sync.dma_start(
                out=xt[:, tj], in_=xT[:, 3072 * tj:3072 * (tj + 1)]
            )
        nc.scalar.dma_start(out=bq_t, in_=bqT[:, :])
        nc.scalar.dma_start(out=bk_t, in_=bkT[:, :])
        nc.gpsimd.dma_start(
            out=bvr,
            in_=bv[:].partition_broadcast(128).rearrange(
                "p (h d) -> p h d", h=NH
            ),
        )

        # warm-up operands for HAM filler matmuls (no DMA dependency)
        warm = consts.tile([128, 512], BF16, tag="warm", name="warm")
        nc.vector.memset(warm, 0.0)
        # all-ones row for the PE-based Z broadcast in emit_norm
        ones64 = consts.tile([1, 64], BF16, tag="o64", name="o64")
        nc.vector.memset(ones64, 1.0)
        # preload the ACT exp table while input DMAs stream
        wexp = consts.tile([128, 1], F32, tag="wexp", name="wexp")
        nc.scalar.activation(wexp, warm[:, 0:1],
                             mybir.ActivationFunctionType.Exp)

        pps = tc.alloc_tile_pool(name="pps", bufs=1, space="PSUM")
        sp = tc.alloc_tile_pool(name="sp", bufs=2, space="PSUM")
        cx = tc.alloc_tile_pool(name="cx", bufs=1, space="PSUM")
        pt16_pool = tc.alloc_tile_pool(name="pt16", bufs=6)
        pt8_pool = tc.alloc_tile_pool(name="pt8", bufs=6)
        npool = tc.alloc_tile_pool(name="np", bufs=3)

        # ---- projection work units (emitted lazily, interleaved with
        # attention so the PE stays dense while ACT chews on exps) ----
        def qk_chain(w_t, b_t, dst, pi, tj):
            def emit():
                ps = pps.tile([128, 512], F32, tag="ps", name="ps")
                for ci in range(NCI):
                    nc.tensor.matmul(
                        ps,
                        w_t[:, ci, 128 * pi:128 * (pi + 1)],
                        xt[:, tj, ci, :],
                        start=(ci == 0),
                        stop=(ci == NCI - 1),
                    )
                nc.vector.tensor_scalar_add(
                    dst[:, pi, 512 * tj:512 * (tj + 1)], ps, b_t[:, pi:pi + 1]
                )
            return emit

        def v_chain(t16):
            def emit():
                ps = pps.tile([128, OC], F32, tag="ps", name="ps")
                for ci in range(NCI):
                    nc.tensor.matmul(
                        ps,
                        xt[:, t16 // 4, ci, 128 * (t16 % 4):128 * (t16 % 4 + 1)],
                        wv_t[:, ci, :],
                        start=(ci == 0),
                        stop=(ci == NCI - 1),
                    )
                nc.vector.tensor_tensor(
                    va16[:, t16, :, 0:D],
                    ps.rearrange("p (h d) -> p h d", h=NH),
                    bvr,
                    op=mybir.AluOpType.add,
                )
                nc.vector.tensor_copy(va16[:, t16, :, D], ones.to_broadcast([128, NH]))
                nc.vector.tensor_copy(va8[:, t16, :, 0:D + 1], va16[:, t16])
            return emit

        chains = {}
        # j=0 units first (smallest data need), then long (ACT-heavy) j=3
        # units interleaved with short ones so exp work spreads out and the
        # projection chains (PE filler) last the whole kernel.
        unit_order = [(0, 0), (0, 1), (0, 2),
                      (1, 0), (3, 0), (1, 1), (2, 0), (3, 1),
                      (1, 2), (2, 1), (3, 2), (2, 2)]
        for pi in range(NPAIR):
            for tj in range(NJ):
                chains[f"q{pi}{tj}"] = qk_chain(wq_t, bq_t, qT, pi, tj)
                chains[f"k{pi}{tj}"] = qk_chain(wk_t, bk_t, kT, pi, tj)
        for t16 in range(NT16):
            chains[f"v{t16}"] = v_chain(t16)

        # q/k chains first needed by each unit (v chains are emitted
        # just-in-time at first ctx use so they pace themselves)
        need = []
        _seen = set()
        for (j, pi) in unit_order:
            lst = []
            for n in [f"q{pi}{j}"] + [f"k{pi}{t}" for t in range(j + 1)]:
                if n not in _seen:
                    _seen.add(n)
                    lst.append(n)
            need.append(lst)
        done = set()

        def emit_chain(name):
            if name not in done:
                done.add(name)
                chains[name]()

        def filler():
            wp = pps.tile([128, 512], F32, tag="ps", name="fil")
            nc.tensor.matmul(wp, warm[:, 0:128], warm,
                             start=True, stop=True)

        # HAM warm-up: keep the PE busy while input DMAs stream in
        for _ in range(20):
            wp = pps.tile([128, 512], F32, tag="ps", name="fil")
            nc.tensor.matmul(wp, warm[:, 0:128], warm,
                             start=True, stop=True)

        # ---- attention ----
        def emit_scores(u):
            kc = u.kc
            u.kc += 1
            c0 = max(0, kc - 4 * u.j) * 128
            emit_chain(f"k{u.pi}{kc // 4}")
            s2 = sp.tile([128, 2, 512], F32, tag="s", name="s2")
            for half in range(2):
                rows = slice(64 * half, 64 * half + 64)
                nc.tensor.matmul(
                    s2[:, half, c0:],
                    kT[rows, u.pi, 128 * kc:128 * (kc + 1)],
                    qT[rows, u.pi, 512 * u.j + c0:512 * (u.j + 1)],
                    start=True, stop=True,
                )
            u.sq.append((kc, c0, s2))

        def emit_exp(u):
            kc, c0, s2 = u.sq.popleft()
            if kc >= 4 * u.j:
                # diagonal chunk: bf16 P, triangular zeroing
                pt = pt16_pool.tile([128, 2, 512], BF16, tag="pt", name="pt")
                nc.scalar.activation(
                    pt[:, :, c0:], s2[:, :, c0:],
                    mybir.ActivationFunctionType.Exp,
                    bias=mk_t[:, kc:kc + 1], scale=0.125,
                )
                for half in range(2):
                    nc.gpsimd.affine_select(
                        out=pt[:, half, c0:c0 + 128],
                        in_=pt[:, half, c0:c0 + 128],
                        compare_op=mybir.AluOpType.is_ge,
                        fill=0.0,
                        base=0,
                        pattern=[[1, 128]],
                        channel_multiplier=-1,
                    )
                u.pend.append(("d", kc, c0, pt))
            else:
                # off-diagonal chunk: fp8 P into the kc-pair slot
                parity = kc % 2
                if parity == 0:
                    u.cur8 = pt8_pool.tile([128, 2, 2, 512], F8,
                                           tag="p8", name="p8")
                nc.scalar.activation(
                    u.cur8[:, parity], s2,
                    mybir.ActivationFunctionType.Exp,
                    bias=mk_t[:, kc:kc + 1], scale=0.125,
                )
                if parity == 1:
                    u.pend.append(("p8", kc - 1, 0, u.cur8))
                    u.cur8 = None

        def emit_step(u, step_i):
            emit_scores(u)
            if len(u.sq) > 1:
                emit_exp(u)
            # paced prefetch of the NEXT unit's projection chains
            if u.pref:
                tgt = (u.pref_n * u.kc + u.nk - 1) // u.nk
                while u.pref and u.pref_done < tgt:
                    emit_chain(u.pref.popleft())
                    u.pref_done += 1
            elif u.tail and step_i % 2 == 0:
                filler()
            if len(u.pend) > 1:
                emit_ctx(u, u.pend.popleft())

        def emit_ctx(u, item):
            kind, kc, c0, pt = item
            emit_chain(f"v{kc}")
            if kind == "p8":
                emit_chain(f"v{kc + 1}")
                for half in range(2):
                    nc.tensor.matmul(
                        u.ctx[half],
                        va8[:, kc:kc + 2, 2 * u.pi + half, 0:D + 1],
                        pt[:, :, half, :],
                        perf_mode=DR,
                        start=(kc == 0),
                        stop=False,
                    )
            else:
                for half in range(2):
                    nc.tensor.matmul(
                        u.ctx[half][:, c0:],
                        va16[:, kc, 2 * u.pi + half, :],
                        pt[:, half, c0:],
                        start=(kc == 0),
                        stop=(kc == u.nk - 1),
                    )

        def emit_norm(u):
            for half in range(2):
                hl = 2 * u.pi + half
                zr = npool.tile([1, 512], F32, tag="zr", name="zr")
                nc.vector.tensor_copy(zr, u.ctx[half][D:D + 1, :])
                zrec = npool.tile([1, 512], F32, tag="zrec", name="zrec")
                nc.vector.reciprocal_approx_fast(zrec, zr)
                zrep = npool.tile([64, 512], F32, tag="zrep", name="zrep")
                nc.gpsimd.partition_broadcast(zrep, zrec)
                ot = npool.tile([64, 512], BF16, tag="ot", name="ot")
                nc.vector.tensor_tensor(
                    ot, u.ctx[half][0:D, :], zrep, op=mybir.AluOpType.mult
                )
                nc.sync.dma_start(
                    out=outT[D * hl:D * (hl + 1), 512 * u.j:512 * (u.j + 1)],
                    in_=ot,
                )

        step_i = 0
        units = [_Unit(pi, j, 0) for (j, pi) in unit_order]
        for i, u in enumerate(units):
            u.pref = deque(need[i + 1]) if i + 1 < len(units) else deque()
            u.pref_n = len(u.pref)
            u.pref_done = 0
            u.tail = i >= len(units) - 2
            for nm in need[i]:
                emit_chain(nm)
            u.ctx = (
                cx.tile([D + 1, 512], F32, tag=f"ca{i % 2}", name="ctxa"),
                cx.tile([D + 1, 512], F32, tag="cb", name="ctxb"),
            )
            while u.kc < u.nk:
                emit_step(u, step_i)
                step_i += 1
            # hide this unit's drain under the next unit's pipeline head
            nxt = units[i + 1] if i + 1 < len(units) else None
            if nxt is not None:
                emit_chain(f"q{nxt.pi}{nxt.j}")
                emit_scores(nxt)
            while u.sq:
                emit_exp(u)
                if nxt is not None and nxt.kc < min(2, nxt.nk):
                    emit_scores(nxt)
            while u.pend:
                emit_ctx(u, u.pend.popleft())
            emit_norm(u)

        npool.release()
        pt8_pool.release()
        pt16_pool.release()
        cx.release()
        sp.release()
        pps.release()
        pin_p.release()
        va_pool.release()
        qk_pool.release()
        consts.release()

    nc.compile()
    return nc


def kernel(**inputs):
    global _COMPILED, LAST_EXEC_NS
    hs = np.asarray(inputs["hidden_states"], dtype=np.float32)
    am = np.asarray(inputs["attention_mask"], dtype=np.float32)
    Wq = np.asarray(inputs["Wq"], dtype=np.float32)
    bq = np.asarray(inputs["bq"], dtype=np.float32)
    Wk = np.asarray(inputs["Wk"], dtype=np.float32)
    bk = np.asarray(inputs["bk"], dtype=np.float32)
    Wv = np.asarray(inputs["Wv"], dtype=np.float32)
    bv = np.asarray(inputs["bv"], dtype=np.float32)

    if _COMPILED is None:
        _COMPILED = _build()
    nc = _COMPILED

    c = np.ascontiguousarray
    bf = ml_dtypes.bfloat16

    def pack_x(xTb):
        # [768, 2048] -> [128, j, ci, 512] flattened, contiguous lines
        return c(xTb.reshape(NCI, 128, NJ, 512).transpose(1, 2, 0, 3)
                 .reshape(128, NJ * NCI * 512))

    def pack_w(WTb):
        # [768, 384] -> [128, ci, 384] flattened
        return c(WTb.reshape(NCI, 128, OC).transpose(1, 0, 2)
                 .reshape(128, NCI * OC))

    in_maps = []
    for core in range(8):
        b, half = core // 2, core % 2
        o0 = OC * half
        sl = slice(o0, o0 + OC)
        in_maps.append({
            "xT": pack_x(hs[b].T.astype(bf)),
            "wqT": pack_w(Wq[sl, :].T.astype(bf)),
            "wkT": pack_w(Wk[sl, :].T.astype(bf)),
            "wvT": pack_w(Wv[sl, :].T.astype(bf)),
            "bqT": c(bq[sl].reshape(NPAIR, 128).T),
            "bkT": c(bk[sl].reshape(NPAIR, 128).T),
            "bv": c(bv[sl]),
            "maskT": c(am[b, 0, 0, :].reshape(NT16, 128).T),
        })

    if _TRACE:
        _install_trace_hook()
    res = run_bass_kernel_spmd(
        nc, in_maps, list(range(8)), trace=_TRACE, tmpdir=_TMPDIR
    )
    LAST_EXEC_NS = res.exec_time_ns

    out = np.empty((B, T, HID), dtype=np.float32)
    for core in range(8):
        b, half = core // 2, core % 2
        out[b, :, OC * half:OC * (half + 1)] = (
            res.results[core]["outT"].astype(np.float32).T
        )
    return out


# revision 41
# speedup vs baseline: 1.0273x; 1.0273x over previous
"""Causal self-attention (B=4, T=2048, HID=768, H=12) on 8 NeuronCores.

Sharding: core c handles batch b=c//2 and head-half c%2 (6 of 12 heads).
Data-parallel on B, tensor-parallel on heads; no cross-device communication.

Per-core kernel:
  - all matmul operands bf16 (PSUM accumulation fp32); P tiles for
    off-diagonal key chunks are fp8e4 and feed DoubleRow ctx matmuls
    (two 128-key chunks contracted per pass, 2x PE throughput there).
    Diagonal chunks stay bf16, so the first 128 queries (tiny softmax
    support, sensitive to value quantization) never touch fp8.
  - host pre-packs x and W so every DMA moves 4-6KB contiguous lines per
    partition (128 descriptors per DMA); DMAs are spread across the
    sync/scalar/gpsimd DGE queues so issue time doesn't serialize.
  - qT/kT laid out [128=2 heads x 64d, token] per pair; scores are
    computed transposed S^T[k, q] per 128-key chunk into [128, 2, 512]
    PSUM; one ACT exp per chunk covers both heads (scale=1/8, additive
    mask as per-partition bias).  Causal masking = column-range
    restriction + gpsimd affine_select on the exp'd diagonal block.
  - V is augmented with a 65th all-ones column so the ctx matmul
    accumulates ctx_num^T = P V and the softmax denominator Z in one
    [65, 512] PSUM tile; normalization = reciprocal_approx_fast ->
    gpsimd partition_broadcast -> DVE multiply -> bf16 output.
  - work units (head-pair, 512-query chunk) run j=0..3 so the x stream
    arrives just ahead of use; filler matmuls keep the PE HAM warm
    through the initial DMA phase.
"""

import sys
from collections import deque

for _p in ("/root/.axon_site/_ro/trn_rl_repo", "/opt/trn_rl_repo"):
    if _p not in sys.path:
        sys.path.append(_p)

import ml_dtypes
import numpy as np

import concourse.bass as bass
import concourse.mybir as mybir
import concourse.tile as tile
from concourse import bacc
from concourse.bass_utils import run_bass_kernel_spmd

F32 = mybir.dt.float32
BF16 = mybir.dt.bfloat16
F8 = mybir.dt.float8e4
DR = mybir.MatmulPerfMode.DoubleRow

B, T, HID, H = 4, 2048, 768, 12
D = HID // H            # 64
NH = 6                  # heads per core
NPAIR = 3               # head pairs per core
OC = NH * D             # 384 output dims per core
NCI = HID // 128        # 6 contraction chunks
NJ = T // 512           # 4 query chunks of 512
NT16 = T // 128         # 16 token chunks of 128

_TRACE = False
_TMPDIR = None
LAST_EXEC_NS = None
_COMPILED = None


def _install_trace_hook():
    import types

    if "antenv.axon_hooks" in sys.modules:
        return
    mod = types.ModuleType("antenv.axon_hooks")
    mod._hook = None
    mod.set_axon_ntff_profile_hook = lambda h: setattr(mod, "_hook", h)
    mod.get_axon_ntff_profile_hook = lambda: mod._hook
    sys.modules["antenv.axon_hooks"] = mod
    sys.path.insert(0, "/root/.axon_site")
    from trn_agent_boot.trn_boot import _ntff_profile_via_ctypes

    mod.set_axon_ntff_profile_hook(
        _ntff_profile_via_ctypes("/opt/axon/libaxon_pjrt.so")
    )


class _Unit:
    """One (head-pair, q-chunk-of-512) attention work unit."""

    def __init__(self, pi, j, slot):
        self.pi = pi
        self.j = j
        self.slot = slot
        self.nk = 4 * (j + 1)
        self.kc = 0
        self.sq = deque()    # scores awaiting exp (1-step delay)
        self.pend = deque()  # exp'd items awaiting ctx (1-step delay)
        self.cur8 = None     # fp8 pt tile collecting the current kc pair
        self.ctx = None


def _build():
    nc = bacc.Bacc("TRN2", target_bir_lowering=False)

    # host pre-packed layouts: per-partition contiguous lines
    xT = nc.dram_tensor("xT", [128, NJ * NCI * 512], BF16, kind="ExternalInput")
    wqT = nc.dram_tensor("wqT", [128, NCI * OC], BF16, kind="ExternalInput")
    wkT = nc.dram_tensor("wkT", [128, NCI * OC], BF16, kind="ExternalInput")
    wvT = nc.dram_tensor("wvT", [128, NCI * OC], BF16, kind="ExternalInput")
    bqT = nc.dram_tensor("bqT", [128, NPAIR], F32, kind="ExternalInput")
    bkT = nc.dram_tensor("bkT", [128, NPAIR], F32, kind="ExternalInput")
    bv = nc.dram_tensor("bv", [OC], F32, kind="ExternalInput")
    maskT = nc.dram_tensor("maskT", [128, NT16], F32, kind="ExternalInput")
    outT = nc.dram_tensor("outT", [OC, T], BF16, kind="ExternalOutput")

    with tile.TileContext(nc) as tc:
        consts = tc.alloc_tile_pool(name="consts", bufs=1)
        qk_pool = tc.alloc_tile_pool(name="qk", bufs=1)
        va_pool = tc.alloc_tile_pool(name="va", bufs=1)

        # ---- constants ----
        bq_t = consts.tile([128, NPAIR], F32, tag="bq")
        bk_t = consts.tile([128, NPAIR], F32, tag="bk")
        bvr = consts.tile([128, NH, D], F32, tag="bvr")
        mk_t = consts.tile([128, NT16], F32, tag="mk")


        # persistent activations
        qT = qk_pool.tile([128, NPAIR, T], BF16, tag="qT")
        kT = qk_pool.tile([128, NPAIR, T], BF16, tag="kT")
        va16 = va_pool.tile([128, NT16, NH, D + 1], BF16, tag="va16")
        # padded to 72B per head so the DoubleRow pair stride (NH*72) is
        # 16B-aligned as checkMatmultPerfMode requires
        va8 = va_pool.tile([128, NT16, NH, 72], F8, tag="va8")
        ones = consts.tile([128, 1], F32, tag="ones", name="ones")
        nc.vector.memset(ones, 1.0)

        pin_p = tc.alloc_tile_pool(name="pin", bufs=1)
        xt = pin_p.tile([128, NJ, NCI, 512], BF16, tag="xt")
        wq_t = pin_p.tile([128, NCI, OC], BF16, tag="wq")
        wk_t = pin_p.tile([128, NCI, OC], BF16, tag="wk")
        wv_t = pin_p.tile([128, NCI, OC], BF16, tag="wv")

        # batched input loads: x chunks on sync, weights on scalar/gpsimd;
        # small tensors issue after the big ones so descriptor generation
        # for the critical weights starts immediately.
        nc.sync.dma_start(out=xt[:, 0], in_=xT[:, 0:3072])
        nc.scalar.dma_start(out=wq_t, in_=wqT[:, :])
        nc.gpsimd.dma_start(out=mk_t, in_=maskT[:, :])
        nc.gpsimd.dma_start(out=wv_t, in_=wvT[:, :])
        nc.scalar.dma_start(out=wk_t, in_=wkT[:, :])
        for tj in (1, 2, 3):
            nc.sync.dma_start(
                out=xt[:, tj], in_=xT[:, 3072 * tj:3072 * (tj + 1)]
            )
        nc.scalar.dma_start(out=bq_t, in_=bqT[:, :])
        nc.scalar.dma_start(out=bk_t, in_=bkT[:, :])
        nc.gpsimd.dma_start(
            out=bvr,
            in_=bv[:].partition_broadcast(128).rearrange(
                "p (h d) -> p h d", h=NH
            ),
        )

        # warm-up operands for HAM filler matmuls (no DMA dependency)
        warm = consts.tile([128, 512], BF16, tag="warm", name="warm")
        nc.vector.memset(warm, 0.0)
        # all-ones row for the PE-based Z broadcast in emit_norm
        ones64 = consts.tile([1, 64], BF16, tag="o64", name="o64")
        nc.vector.memset(ones64, 1.0)
        # preload the ACT exp table while input DMAs stream
        wexp = consts.tile([128, 1], F32, tag="wexp", name="wexp")
        nc.scalar.activation(wexp, warm[:, 0:1],
                             mybir.ActivationFunctionType.Exp)

        pps = tc.alloc_tile_pool(name="pps", bufs=1, space="PSUM")
        sp = tc.alloc_tile_pool(name="sp", bufs=2, space="PSUM")
        cx = tc.alloc_tile_pool(name="cx", bufs=1, space="PSUM")
        pt16_pool = tc.alloc_tile_pool(name="pt16", bufs=6)
        pt8_pool = tc.alloc_tile_pool(name="pt8", bufs=6)
        npool = tc.alloc_tile_pool(name="np", bufs=3)

        # ---- projection work units (emitted lazily, interleaved with
        # attention so the PE stays dense while ACT chews on exps) ----
        def qk_chain(w_t, b_t, dst, pi, tj):
            def emit():
                ps = pps.tile([128, 512], F32, tag="ps", name="ps")
                for ci in range(NCI):
                    nc.tensor.matmul(
                        ps,
                        w_t[:, ci, 128 * pi:128 * (pi + 1)],
                        xt[:, tj, ci, :],
                        start=(ci == 0),
                        stop=(ci == NCI - 1),
                    )
                nc.vector.tensor_scalar_add(
                    dst[:, pi, 512 * tj:512 * (tj + 1)], ps, b_t[:, pi:pi + 1]
                )
            return emit

        def v_chain(t16):
            def emit():
                ps = pps.tile([128, OC], F32, tag="ps", name="ps")
                for ci in range(NCI):
                    nc.tensor.matmul(
                        ps,
                        xt[:, t16 // 4, ci, 128 * (t16 % 4):128 * (t16 % 4 + 1)],
                        wv_t[:, ci, :],
                        start=(ci == 0),
                        stop=(ci == NCI - 1),
                    )
                nc.vector.tensor_tensor(
                    va16[:, t16, :, 0:D],
                    ps.rearrange("p (h d) -> p h d", h=NH),
                    bvr,
                    op=mybir.AluOpType.add,
                )
                nc.vector.tensor_copy(va16[:, t16, :, D], ones.to_broadcast([128, NH]))
                nc.vector.tensor_copy(va8[:, t16, :, 0:D + 1], va16[:, t16])
            return emit

        chains = {}
        # j=0 units first (smallest data need), then long (ACT-heavy) j=3
        # units interleaved with short ones so exp work spreads out and the
        # projection chains (PE filler) last the whole kernel.
        unit_order = [(0, 0), (0, 1), (0, 2),
                      (1, 0), (3, 0), (1, 1), (2, 0), (3, 1),
                      (1, 2), (2, 1), (3, 2), (2, 2)]
        for pi in range(NPAIR):
            for tj in range(NJ):
                chains[f"q{pi}{tj}"] = qk_chain(wq_t, bq_t, qT, pi, tj)
                chains[f"k{pi}{tj}"] = qk_chain(wk_t, bk_t, kT, pi, tj)
        for t16 in range(NT16):
            chains[f"v{t16}"] = v_chain(t16)

        # chains first needed by each unit, in need order
        need = []
        _seen = set()
        for (j, pi) in unit_order:
            lst = []
            for n in ([f"q{pi}{j}"] + [f"k{pi}{t}" for t in range(j + 1)]
                      + [f"v{t}" for t in range(4 * (j + 1))]):
                if n not in _seen:
                    _seen.add(n)
                    lst.append(n)
            need.append(lst)
        done = set()

        def emit_chain(name):
            if name not in done:
                done.add(name)
                chains[name]()

        def filler():
            wp = pps.tile([128, 512], F32, tag="ps", name="fil")
            nc.tensor.matmul(wp, warm[:, 0:128], warm,
                             start=True, stop=True)

        # HAM warm-up: keep the PE busy while input DMAs stream in
        for _ in range(20):
            wp = pps.tile([128, 512], F32, tag="ps", name="fil")
            nc.tensor.matmul(wp, warm[:, 0:128], warm,
                             start=True, stop=True)

        # ---- attention ----
        def emit_scores(u):
            kc = u.kc
            u.kc += 1
            c0 = max(0, kc - 4 * u.j) * 128
            emit_chain(f"k{u.pi}{kc // 4}")
            s2 = sp.tile([128, 2, 512], F32, tag="s", name="s2")
            for half in range(2):
                rows = slice(64 * half, 64 * half + 64)
                nc.tensor.matmul(
                    s2[:, half, c0:],
                    kT[rows, u.pi, 128 * kc:128 * (kc + 1)],
                    qT[rows, u.pi, 512 * u.j + c0:512 * (u.j + 1)],
                    start=True, stop=True,
                )
            u.sq.append((kc, c0, s2))

        def emit_exp(u):
            kc, c0, s2 = u.sq.popleft()
            if kc >= 4 * u.j:
                # diagonal chunk: bf16 P, triangular zeroing
                pt = pt16_pool.tile([128, 2, 512], BF16, tag="pt", name="pt")
                nc.scalar.activation(
                    pt[:, :, c0:], s2[:, :, c0:],
                    mybir.ActivationFunctionType.Exp,
                    bias=mk_t[:, kc:kc + 1], scale=0.125,
                )
                for half in range(2):
                    nc.gpsimd.affine_select(
                        out=pt[:, half, c0:c0 + 128],
                        in_=pt[:, half, c0:c0 + 128],
                        compare_op=mybir.AluOpType.is_ge,
                        fill=0.0,
                        base=0,
                        pattern=[[1, 128]],
                        channel_multiplier=-1,
                    )
                u.pend.append(("d", kc, c0, pt))
            else:
                # off-diagonal chunk: fp8 P into the kc-pair slot
                parity = kc % 2
                if parity == 0:
                    u.cur8 = pt8_pool.tile([128, 2, 2, 512], F8,
                                           tag="p8", name="p8")
                nc.scalar.activation(
                    u.cur8[:, parity], s2,
                    mybir.ActivationFunctionType.Exp,
                    bias=mk_t[:, kc:kc + 1], scale=0.125,
                )
                if parity == 1:
                    u.pend.append(("p8", kc - 1, 0, u.cur8))
                    u.cur8 = None

        def emit_step(u, step_i):
            emit_scores(u)
            if len(u.sq) > 1:
                emit_exp(u)
            # paced prefetch of the NEXT unit's projection chains
            if u.pref:
                tgt = (u.pref_n * u.kc + u.nk - 1) // u.nk
                while u.pref and u.pref_done < tgt:
                    emit_chain(u.pref.popleft())
                    u.pref_done += 1
            elif u.tail and step_i % 2 == 0:
                filler()
            if len(u.pend) > 1:
                emit_ctx(u, u.pend.popleft())

        def emit_ctx(u, item):
            kind, kc, c0, pt = item
            emit_chain(f"v{kc}")
            if kind == "p8":
                emit_chain(f"v{kc + 1}")
                for half in range(2):
                    nc.tensor.matmul(
                        u.ctx[half],
                        va8[:, kc:kc + 2, 2 * u.pi + half, 0:D + 1],
                        pt[:, :, half, :],
                        perf_mode=DR,
                        start=(kc == 0),
                        stop=False,
                    )
            else:
                for half in range(2):
                    nc.tensor.matmul(
                        u.ctx[half][:, c0:],
                        va16[:, kc, 2 * u.pi + half, :],
                        pt[:, half, c0:],
                        start=(kc == 0),
                        stop=(kc == u.nk - 1),
                    )

        def emit_norm(u):
            for half in range(2):
                hl = 2 * u.pi + half
                zr = npool.tile([1, 512], F32, tag="zr", name="zr")
                nc.vector.tensor_copy(zr, u.ctx[half][D:D + 1, :])
                zrec = npool.tile([1, 512], F32, tag="zrec", name="zrec")
                nc.vector.reciprocal_approx_fast(zrec, zr)
                zrep = npool.tile([64, 512], F32, tag="zrep", name="zrep")
                nc.gpsimd.partition_broadcast(zrep, zrec)
                ot = npool.tile([64, 512], BF16, tag="ot", name="ot")
                nc.vector.tensor_tensor(
                    ot, u.ctx[half][0:D, :], zrep, op=mybir.AluOpType.mult
                )
                nc.sync.dma_start(
                    out=outT[D * hl:D * (hl + 1), 512 * u.j:512 * (u.j + 1)],
                    in_=ot,
                )

        step_i = 0
        units = [_Unit(pi, j, 0) for (j, pi) in unit_order]
        for i, u in enumerate(units):
            u.pref = deque(need[i + 1]) if i + 1 < len(units) else deque()
            u.pref_n = len(u.pref)
            u.pref_done = 0
            u.tail = i >= len(units) - 2
            for nm in need[i]:
                emit_chain(nm)
            u.ctx = (
                cx.tile([D + 1, 512], F32, tag=f"ca{i % 2}", name="ctxa"),
                cx.tile([D + 1, 512], F32, tag="cb", name="ctxb"),
            )
            while u.kc < u.nk:
                emit_step(u, step_i)
                step_i += 1
            # hide this unit's drain under the next unit's pipeline head
            nxt = units[i + 1] if i + 1 < len(units) else None
            if nxt is not None:
                emit_chain(f"q{nxt.pi}{nxt.j}")
                emit_scores(nxt)
            while u.sq:
                emit_exp(u)
                if nxt is not None and nxt.kc < min(2, nxt.nk):
                    emit_scores(nxt)
            while u.pend:
                emit_ctx(u, u.pend.popleft())
            emit_norm(u)

        npool.release()
        pt8_pool.release()
        pt16_pool.release()
        cx.release()
        sp.release()
        pps.release()
        pin_p.release()
        va_pool.release()
        qk_pool.release()
        consts.release()

    nc.compile()
    return nc


def kernel(**inputs):
    global _COMPILED, LAST_EXEC_NS
    hs = np.asarray(inputs["hidden_states"], dtype=np.float32)
    am = np.asarray(inputs["attention_mask"], dtype=np.float32)
    Wq = np.asarray(inputs["Wq"], dtype=np.float32)
    bq = np.asarray(inputs["bq"], dtype=np.float32)
    Wk = np.asarray(inputs["Wk"], dtype=np.float32)
    bk = np.asarray(inputs["bk"], dtype=np.float32)
    Wv = np.asarray(inputs["Wv"], dtype=np.float32)
    bv = np.asarray(inputs["bv"], dtype=np.float32)

    if _COMPILED is None:
        _COMPILED = _build()
    nc = _COMPILED

    c = np.ascontiguousarray
    bf = ml_dtypes.bfloat16

    def pack_x(xTb):
        # [768, 2048] -> [128, j, ci, 512] flattened, contiguous lines
        return c(xTb.reshape(NCI, 128, NJ, 512).transpose(1, 2, 0, 3)
                 .reshape(128, NJ * NCI * 512))

    def pack_w(WTb):
        # [768, 384] -> [128, ci, 384] flattened
        return c(WTb.reshape(NCI, 128, OC).transpose(1, 0, 2)
                 .reshape(128, NCI * OC))

    in_maps = []
    for core in range(8):
        b, half = core // 2, core % 2
        o0 = OC * half
        sl = slice(o0, o0 + OC)
        in_maps.append({
            "xT": pack_x(hs[b].T.astype(bf)),
            "wqT": pack_w(Wq[sl, :].T.astype(bf)),
            "wkT": pack_w(Wk[sl, :].T.astype(bf)),
            "wvT": pack_w(Wv[sl, :].T.astype(bf)),
            "bqT": c(bq[sl].reshape(NPAIR, 128).T),
            "bkT": c(bk[sl].reshape(NPAIR, 128).T),
            "bv": c(bv[sl]),
            "maskT": c(am[b, 0, 0, :].reshape(NT16, 128).T),
        })

    if _TRACE:
        _install_trace_hook()
    res = run_bass_kernel_spmd(
        nc, in_maps, list(range(8)), trace=_TRACE, tmpdir=_TMPDIR
    )
    LAST_EXEC_NS = res.exec_time_ns

    out = np.empty((B, T, HID), dtype=np.float32)
    for core in range(8):
        b, half = core // 2, core % 2
        out[b, :, OC * half:OC * (half + 1)] = (
            res.results[core]["outT"].astype(np.float32).T
        )
    return out


# revision 57
# speedup vs baseline: 1.0291x; 1.0018x over previous
"""Causal self-attention (B=4, T=2048, HID=768, H=12) on 8 NeuronCores.

Sharding: core c handles batch b=c//2 and head-half c%2 (6 of 12 heads).
Data-parallel on B, tensor-parallel on heads; no cross-device communication.

Per-core kernel:
  - all matmul operands bf16 (PSUM accumulation fp32); P tiles for
    off-diagonal key chunks are fp8e4 and feed DoubleRow ctx matmuls
    (two 128-key chunks contracted per pass, 2x PE throughput there).
    Diagonal chunks stay bf16, so the first 128 queries (tiny softmax
    support, sensitive to value quantization) never touch fp8.
  - host pre-packs x and W so every DMA moves 4-6KB contiguous lines per
    partition (128 descriptors per DMA); DMAs are spread across the
    sync/scalar/gpsimd DGE queues so issue time doesn't serialize.
  - qT/kT laid out [128=2 heads x 64d, token] per pair; scores are
    computed transposed S^T[k, q] per 128-key chunk into [128, 2, 512]
    PSUM; one ACT exp per chunk covers both heads (scale=1/8, additive
    mask as per-partition bias).  Causal masking = column-range
    restriction + gpsimd affine_select on the exp'd diagonal block.
  - V is augmented with a 65th all-ones column so the ctx matmul
    accumulates ctx_num^T = P V and the softmax denominator Z in one
    [65, 512] PSUM tile; normalization = reciprocal_approx_fast ->
    gpsimd partition_broadcast -> DVE multiply -> bf16 output.
  - work units (head-pair, 512-query chunk) run j=0..3 so the x stream
    arrives just ahead of use; filler matmuls keep the PE HAM warm
    through the initial DMA phase.
"""

import sys
from collections import deque

for _p in ("/root/.axon_site/_ro/trn_rl_repo", "/opt/trn_rl_repo"):
    if _p not in sys.path:
        sys.path.append(_p)

import ml_dtypes
import numpy as np

import concourse.bass as bass
import concourse.mybir as mybir
import concourse.tile as tile
from concourse import bacc
from concourse.bass_utils import run_bass_kernel_spmd

F32 = mybir.dt.float32
BF16 = mybir.dt.bfloat16
F8 = mybir.dt.float8e4
DR = mybir.MatmulPerfMode.DoubleRow

B, T, HID, H = 4, 2048, 768, 12
D = HID // H            # 64
NH = 6                  # heads per core
NPAIR = 3               # head pairs per core
OC = NH * D             # 384 output dims per core
NCI = HID // 128        # 6 contraction chunks
NJ = T // 512           # 4 query chunks of 512
NT16 = T // 128         # 16 token chunks of 128

_TRACE = False
_TMPDIR = None
LAST_EXEC_NS = None
_COMPILED = None


def _install_trace_hook():
    import types

    if "antenv.axon_hooks" in sys.modules:
        return
    mod = types.ModuleType("antenv.axon_hooks")
    mod._hook = None
    mod.set_axon_ntff_profile_hook = lambda h: setattr(mod, "_hook", h)
    mod.get_axon_ntff_profile_hook = lambda: mod._hook
    sys.modules["antenv.axon_hooks"] = mod
    sys.path.insert(0, "/root/.axon_site")
    from trn_agent_boot.trn_boot import _ntff_profile_via_ctypes

    mod.set_axon_ntff_profile_hook(
        _ntff_profile_via_ctypes("/opt/axon/libaxon_pjrt.so")
    )


class _Unit:
    """One (head-pair, q-chunk-of-512) attention work unit."""

    def __init__(self, pi, j, slot):
        self.pi = pi
        self.j = j
        self.slot = slot
        self.nk = 4 * (j + 1)
        self.kc = 0
        self.sq = deque()    # scores awaiting exp (1-step delay)
        self.pend = deque()  # exp'd items awaiting ctx (1-step delay)
        self.cur8 = None     # fp8 pt tile collecting the current kc pair
        self.ctx = None


def _build():
    nc = bacc.Bacc("TRN2", target_bir_lowering=False)

    # host pre-packed layouts: per-partition contiguous lines
    xT = nc.dram_tensor("xT", [128, NJ * NCI * 512], BF16, kind="ExternalInput")
    # wq/wk packed per head-pair: [128, pair, ci, 128] so the first unit's
    # weight slices can stream ahead of the rest
    wqT = nc.dram_tensor("wqT", [128, NPAIR * NCI * 128], BF16, kind="ExternalInput")
    wkT = nc.dram_tensor("wkT", [128, NPAIR * NCI * 128], BF16, kind="ExternalInput")
    wvT = nc.dram_tensor("wvT", [128, NCI * OC], BF16, kind="ExternalInput")
    bqT = nc.dram_tensor("bqT", [128, NPAIR], F32, kind="ExternalInput")
    bkT = nc.dram_tensor("bkT", [128, NPAIR], F32, kind="ExternalInput")
    bv = nc.dram_tensor("bv", [OC], F32, kind="ExternalInput")
    maskT = nc.dram_tensor("maskT", [128, NT16], F32, kind="ExternalInput")
    outT = nc.dram_tensor("outT", [OC, T], BF16, kind="ExternalOutput")

    with tile.TileContext(nc) as tc:
        consts = tc.alloc_tile_pool(name="consts", bufs=1)
        qk_pool = tc.alloc_tile_pool(name="qk", bufs=1)
        va_pool = tc.alloc_tile_pool(name="va", bufs=1)

        # ---- constants ----
        bq_t = consts.tile([128, NPAIR], F32, tag="bq")
        bk_t = consts.tile([128, NPAIR], F32, tag="bk")
        bvr = consts.tile([128, NH, D], F32, tag="bvr")
        mk_t = consts.tile([128, NT16], F32, tag="mk")


        # persistent activations
        qT = qk_pool.tile([128, NPAIR, T], BF16, tag="qT")
        kT = qk_pool.tile([128, NPAIR, T], BF16, tag="kT")
        va16 = va_pool.tile([128, NT16, NH, D + 1], BF16, tag="va16")
        # padded to 72B per head so the DoubleRow pair stride (NH*72) is
        # 16B-aligned as checkMatmultPerfMode requires
        va8 = va_pool.tile([128, NT16, NH, 72], F8, tag="va8")
        ones = consts.tile([128, 1], F32, tag="ones", name="ones")
        nc.vector.memset(ones, 1.0)

        pin_p = tc.alloc_tile_pool(name="pin", bufs=1)
        xt = pin_p.tile([128, NJ, NCI, 512], BF16, tag="xt")
        wq_t = pin_p.tile([128, NPAIR, NCI, 128], BF16, tag="wq")
        wk_t = pin_p.tile([128, NPAIR, NCI, 128], BF16, tag="wk")
        wv_t = pin_p.tile([128, NCI, OC], BF16, tag="wv")

        # batched input loads: x chunks on sync, weights on scalar/gpsimd;
        # small tensors issue after the big ones so descriptor generation
        # for the critical weights starts immediately.
        PW = NCI * 128
        nc.sync.dma_start(out=xt[:, 0], in_=xT[:, 0:3072])
        nc.scalar.dma_start(out=wq_t[:, 0], in_=wqT[:, 0:PW])
        nc.scalar.dma_start(out=wk_t[:, 0], in_=wkT[:, 0:PW])
        nc.gpsimd.dma_start(out=mk_t, in_=maskT[:, :])
        nc.gpsimd.dma_start(out=wv_t, in_=wvT[:, :])
        nc.scalar.dma_start(out=wq_t[:, 1:3], in_=wqT[:, PW:3 * PW])
        nc.scalar.dma_start(out=wk_t[:, 1:3], in_=wkT[:, PW:3 * PW])
        for tj in (1, 2, 3):
            nc.sync.dma_start(
                out=xt[:, tj], in_=xT[:, 3072 * tj:3072 * (tj + 1)]
            )
        nc.scalar.dma_start(out=bq_t, in_=bqT[:, :])
        nc.scalar.dma_start(out=bk_t, in_=bkT[:, :])
        nc.gpsimd.dma_start(
            out=bvr,
            in_=bv[:].partition_broadcast(128).rearrange(
                "p (h d) -> p h d", h=NH
            ),
        )

        # warm-up operands for HAM filler matmuls (no DMA dependency)
        warm = consts.tile([128, 512], BF16, tag="warm", name="warm")
        nc.vector.memset(warm, 0.0)
        # all-ones row for the PE-based Z broadcast in emit_norm
        ones64 = consts.tile([1, 64], BF16, tag="o64", name="o64")
        nc.vector.memset(ones64, 1.0)
        # preload the ACT exp table while input DMAs stream
        wexp = consts.tile([128, 1], F32, tag="wexp", name="wexp")
        nc.scalar.activation(wexp, warm[:, 0:1],
                             mybir.ActivationFunctionType.Exp)

        pps = tc.alloc_tile_pool(name="pps", bufs=1, space="PSUM")
        sp = tc.alloc_tile_pool(name="sp", bufs=2, space="PSUM")
        cx = tc.alloc_tile_pool(name="cx", bufs=1, space="PSUM")
        pt16_pool = tc.alloc_tile_pool(name="pt16", bufs=6)
        pt8_pool = tc.alloc_tile_pool(name="pt8", bufs=6)
        npool = tc.alloc_tile_pool(name="np", bufs=3)

        # ---- projection work units (emitted lazily, interleaved with
        # attention so the PE stays dense while ACT chews on exps) ----
        def qk_chain(w_t, b_t, dst, pi, tj):
            def emit():
                ps = pps.tile([128, 512], F32, tag="ps", name="ps")
                for ci in range(NCI):
                    nc.tensor.matmul(
                        ps,
                        w_t[:, pi, ci, :],
                        xt[:, tj, ci, :],
                        start=(ci == 0),
                        stop=(ci == NCI - 1),
                    )
                nc.vector.tensor_scalar_add(
                    dst[:, pi, 512 * tj:512 * (tj + 1)], ps, b_t[:, pi:pi + 1]
                )
            return emit

        def v_chain(t16):
            def emit():
                ps = pps.tile([128, OC], F32, tag="ps", name="ps")
                tj, t0 = t16 // 4, 128 * (t16 % 4)
                for ci in range(NCI):
                    nc.tensor.matmul(
                        ps, xt[:, tj, ci, t0:t0 + 128], wv_t[:, ci, :],
                        start=(ci == 0), stop=(ci == NCI - 1),
                    )
                nc.vector.tensor_tensor(
                    va16[:, t16, :, 0:D],
                    ps.rearrange("p (h d) -> p h d", h=NH),
                    bvr,
                    op=mybir.AluOpType.add,
                )
                nc.vector.tensor_copy(va16[:, t16, :, D], ones.to_broadcast([128, NH]))
                nc.vector.tensor_copy(va8[:, t16, :, 0:D + 1], va16[:, t16])
            return emit

        chains = {}
        # j=0 units first (smallest data need), then long (ACT-heavy) j=3
        # units interleaved with short ones so exp work spreads out and the
        # projection chains (PE filler) last the whole kernel.
        unit_order = [(0, 0), (0, 1), (0, 2),
                      (1, 0), (3, 0), (1, 1), (2, 0), (3, 1),
                      (1, 2), (2, 1), (3, 2), (2, 2)]
        for pi in range(NPAIR):
            for tj in range(NJ):
                chains[f"q{pi}{tj}"] = qk_chain(wq_t, bq_t, qT, pi, tj)
                chains[f"k{pi}{tj}"] = qk_chain(wk_t, bk_t, kT, pi, tj)
        for t16 in range(NT16):
            chains[f"v{t16}"] = v_chain(t16)

        # chains first needed by each unit, in need order
        need = []
        _seen = set()
        for (j, pi) in unit_order:
            lst = []
            for n in ([f"q{pi}{j}"] + [f"k{pi}{t}" for t in range(j + 1)]
                      + [f"v{t}" for t in range(4 * (j + 1))]):
                if n not in _seen:
                    _seen.add(n)
                    lst.append(n)
            need.append(lst)
        gall = deque(n for lst in need for n in lst)
        done = set()

        def emit_chain(name):
            if name not in done:
                done.add(name)
                chains[name]()

        def filler():
            wp = pps.tile([128, 512], F32, tag="ps", name="fil")
            nc.tensor.matmul(wp, warm[:, 0:128], warm,
                             start=True, stop=True)

        # HAM warm-up: keep the PE busy while input DMAs stream in
        for _ in range(12):
            wp = pps.tile([128, 512], F32, tag="ps", name="fil")
            nc.tensor.matmul(wp, warm[:, 0:128], warm,
                             start=True, stop=True)

        # ---- attention ----
        def emit_scores(u):
            kc = u.kc
            u.kc += 1
            c0 = max(0, kc - 4 * u.j) * 128
            emit_chain(f"k{u.pi}{kc // 4}")
            s2 = sp.tile([128, 2, 512], F32, tag="s", name="s2")
            for half in range(2):
                rows = slice(64 * half, 64 * half + 64)
                nc.tensor.matmul(
                    s2[:, half, c0:],
                    kT[rows, u.pi, 128 * kc:128 * (kc + 1)],
                    qT[rows, u.pi, 512 * u.j + c0:512 * (u.j + 1)],
                    start=True, stop=True,
                )
            u.sq.append((kc, c0, s2))

        def emit_exp(u):
            kc, c0, s2 = u.sq.popleft()
            if kc >= 4 * u.j:
                # diagonal chunk: bf16 P, triangular zeroing
                pt = pt16_pool.tile([128, 2, 512], BF16, tag="pt", name="pt")
                nc.scalar.activation(
                    pt[:, :, c0:], s2[:, :, c0:],
                    mybir.ActivationFunctionType.Exp,
                    bias=mk_t[:, kc:kc + 1], scale=0.125,
                )
                for half in range(2):
                    nc.gpsimd.affine_select(
                        out=pt[:, half, c0:c0 + 128],
                        in_=pt[:, half, c0:c0 + 128],
                        compare_op=mybir.AluOpType.is_ge,
                        fill=0.0,
                        base=0,
                        pattern=[[1, 128]],
                        channel_multiplier=-1,
                    )
                u.pend.append(("d", kc, c0, pt))
            else:
                # off-diagonal chunk: fp8 P into the kc-pair slot
                parity = kc % 2
                if parity == 0:
                    u.cur8 = pt8_pool.tile([128, 2, 2, 512], F8,
                                           tag="p8", name="p8")
                nc.scalar.activation(
                    u.cur8[:, parity], s2,
                    mybir.ActivationFunctionType.Exp,
                    bias=mk_t[:, kc:kc + 1], scale=0.125,
                )
                if parity == 1:
                    u.pend.append(("p8", kc - 1, 0, u.cur8))
                    u.cur8 = None

        def emit_step(u, step_i):
            emit_scores(u)
            if len(u.sq) > 1:
                emit_exp(u)
            # paced prefetch of the NEXT unit's projection chains
            if u.pref:
                tgt = (u.pref_n * u.kc + u.nk - 1) // u.nk
                while u.pref and u.pref_done < tgt:
                    emit_chain(u.pref.popleft())
                    u.pref_done += 1
            elif u.tail and step_i % 2 == 0:
                filler()
            if u.j == 0:
                # input-bound phase: PE has slack, pull chains forward
                while gall and gall[0] in done:
                    gall.popleft()
                if gall:
                    emit_chain(gall.popleft())
            if len(u.pend) > 1:
                emit_ctx(u, u.pend.popleft())

        def emit_ctx(u, item):
            kind, kc, c0, pt = item
            emit_chain(f"v{kc}")
            if kind == "p8":
                emit_chain(f"v{kc + 1}")
                for half in range(2):
                    nc.tensor.matmul(
                        u.ctx[half],
                        va8[:, kc:kc + 2, 2 * u.pi + half, 0:D + 1],
                        pt[:, :, half, :],
                        perf_mode=DR,
                        start=(kc == 0),
                        stop=False,
                    )
            else:
                for half in range(2):
                    nc.tensor.matmul(
                        u.ctx[half][:, c0:],
                        va16[:, kc, 2 * u.pi + half, :],
                        pt[:, half, c0:],
                        start=(kc == 0),
                        stop=(kc == u.nk - 1),
                    )

        def emit_norm(u):
            for half in range(2):
                hl = 2 * u.pi + half
                zr = npool.tile([1, 512], F32, tag="zr", name="zr")
                nc.vector.tensor_copy(zr, u.ctx[half][D:D + 1, :])
                zrec = npool.tile([1, 512], F32, tag="zrec", name="zrec")
                nc.vector.reciprocal_approx_fast(zrec, zr)
                zrep = npool.tile([64, 512], F32, tag="zrep", name="zrep")
                nc.gpsimd.partition_broadcast(zrep, zrec)
                ot = npool.tile([64, 512], BF16, tag="ot", name="ot")
                nc.vector.tensor_tensor(
                    ot, u.ctx[half][0:D, :], zrep, op=mybir.AluOpType.mult
                )
                nc.sync.dma_start(
                    out=outT[D * hl:D * (hl + 1), 512 * u.j:512 * (u.j + 1)],
                    in_=ot,
                )

        step_i = 0
        units = [_Unit(pi, j, 0) for (j, pi) in unit_order]
        for i, u in enumerate(units):
            u.pref = deque(need[i + 1]) if i + 1 < len(units) else deque()
            u.pref_n = len(u.pref)
            u.pref_done = 0
            u.tail = i >= len(units) - 2
            for nm in need[i]:
                emit_chain(nm)
            u.ctx = (
                cx.tile([D + 1, 512], F32, tag=f"ca{i % 2}", name="ctxa"),
                cx.tile([D + 1, 512], F32, tag="cb", name="ctxb"),
            )
            while u.kc < u.nk:
                emit_step(u, step_i)
                step_i += 1
            # hide this unit's drain under the next unit's pipeline head
            nxt = units[i + 1] if i + 1 < len(units) else None
            if nxt is not None:
                emit_chain(f"q{nxt.pi}{nxt.j}")
                emit_scores(nxt)
            while u.sq:
                emit_exp(u)
                if nxt is not None and nxt.kc < min(2, nxt.nk):
                    emit_scores(nxt)
            while u.pend:
                emit_ctx(u, u.pend.popleft())
            emit_norm(u)

        npool.release()
        pt8_pool.release()
        pt16_pool.release()
        cx.release()
        sp.release()
        pps.release()
        pin_p.release()
        va_pool.release()
        qk_pool.release()
        consts.release()

    nc.compile()
    return nc


def kernel(**inputs):
    global _COMPILED, LAST_EXEC_NS
    hs = np.asarray(inputs["hidden_states"], dtype=np.float32)
    am = np.asarray(inputs["attention_mask"], dtype=np.float32)
    Wq = np.asarray(inputs["Wq"], dtype=np.float32)
    bq = np.asarray(inputs["bq"], dtype=np.float32)
    Wk = np.asarray(inputs["Wk"], dtype=np.float32)
    bk = np.asarray(inputs["bk"], dtype=np.float32)
    Wv = np.asarray(inputs["Wv"], dtype=np.float32)
    bv = np.asarray(inputs["bv"], dtype=np.float32)

    if _COMPILED is None:
        _COMPILED = _build()
    nc = _COMPILED

    c = np.ascontiguousarray
    bf = ml_dtypes.bfloat16
    f8 = ml_dtypes.float8_e4m3

    def pack_x(xTb):
        # [768, 2048] -> [128, j, ci, 512] flattened, contiguous lines
        return c(xTb.reshape(NCI, 128, NJ, 512).transpose(1, 2, 0, 3)
                 .reshape(128, NJ * NCI * 512))

    def pack_w(WTb):
        # [768, 384] -> [128, ci, 384] flattened
        return c(WTb.reshape(NCI, 128, OC).transpose(1, 0, 2)
                 .reshape(128, NCI * OC))

    def pack_w_pair(WTb):
        # [768, 384] -> [128, pair, ci, 128] flattened
        return c(WTb.reshape(NCI, 128, NPAIR, 128).transpose(1, 2, 0, 3)
                 .reshape(128, NPAIR * NCI * 128))

    in_maps = []
    for core in range(8):
        b, half = core // 2, core % 2
        o0 = OC * half
        sl = slice(o0, o0 + OC)
        in_maps.append({
            "xT": pack_x(hs[b].T.astype(bf)),
            "wqT": pack_w_pair(Wq[sl, :].T.astype(bf)),
            "wkT": pack_w_pair(Wk[sl, :].T.astype(bf)),
            "wvT": pack_w(Wv[sl, :].T.astype(bf)),
            "bqT": c(bq[sl].reshape(NPAIR, 128).T),
            "bkT": c(bk[sl].reshape(NPAIR, 128).T),
            "bv": c(bv[sl]),
            "maskT": c(am[b, 0, 0, :].reshape(NT16, 128).T),
        })

    if _TRACE:
        _install_trace_hook()
    res = run_bass_kernel_spmd(
        nc, in_maps, list(range(8)), trace=_TRACE, tmpdir=_TMPDIR
    )
    LAST_EXEC_NS = res.exec_time_ns

    out = np.empty((B, T, HID), dtype=np.float32)
    for core in range(8):
        b, half = core // 2, core % 2
        out[b, :, OC * half:OC * (half + 1)] = (
            res.results[core]["outT"].astype(np.float32).T
        )
    return out


# revision 65
# speedup vs baseline: 1.0420x; 1.0126x over previous
"""Causal self-attention (B=4, T=2048, HID=768, H=12) on 8 NeuronCores.

Sharding: core c handles batch b=c//2 and head-half c%2 (6 of 12 heads).
Data-parallel on B, tensor-parallel on heads; no cross-device communication.

Per-core kernel:
  - all matmul operands bf16 (PSUM accumulation fp32); P tiles for
    off-diagonal key chunks are fp8e4 and feed DoubleRow ctx matmuls
    (two 128-key chunks contracted per pass, 2x PE throughput there).
    Diagonal chunks stay bf16, so the first 128 queries (tiny softmax
    support, sensitive to value quantization) never touch fp8.
  - host pre-packs x and W so every DMA moves 4-6KB contiguous lines per
    partition (128 descriptors per DMA); DMAs are spread across the
    sync/scalar/gpsimd DGE queues so issue time doesn't serialize.
  - qT/kT laid out [128=2 heads x 64d, token] per pair; scores are
    computed transposed S^T[k, q] per 128-key chunk into [128, 2, 512]
    PSUM; one ACT exp per chunk covers both heads (scale=1/8, additive
    mask as per-partition bias).  Causal masking = column-range
    restriction + gpsimd affine_select on the exp'd diagonal block.
  - V is augmented with a 65th all-ones column so the ctx matmul
    accumulates ctx_num^T = P V and the softmax denominator Z in one
    [65, 512] PSUM tile; normalization = reciprocal_approx_fast ->
    gpsimd partition_broadcast -> DVE multiply -> bf16 output.
  - work units (head-pair, 512-query chunk) run j=0..3 so the x stream
    arrives just ahead of use; filler matmuls keep the PE HAM warm
    through the initial DMA phase.
"""

import sys
from collections import deque

for _p in ("/root/.axon_site/_ro/trn_rl_repo", "/opt/trn_rl_repo"):
    if _p not in sys.path:
        sys.path.append(_p)

import ml_dtypes
import numpy as np

import concourse.bass as bass
import concourse.mybir as mybir
import concourse.tile as tile
from concourse import bacc
from concourse.bass_utils import run_bass_kernel_spmd

F32 = mybir.dt.float32
BF16 = mybir.dt.bfloat16
F8 = mybir.dt.float8e4
DR = mybir.MatmulPerfMode.DoubleRow

B, T, HID, H = 4, 2048, 768, 12
D = HID // H            # 64
NH = 6                  # heads per core
NPAIR = 3               # head pairs per core
OC = NH * D             # 384 output dims per core
NCI = HID // 128        # 6 contraction chunks
NJ = T // 512           # 4 query chunks of 512
NT16 = T // 128         # 16 token chunks of 128

_TRACE = False
_TMPDIR = None
LAST_EXEC_NS = None
_COMPILED = None


def _install_trace_hook():
    import types

    if "antenv.axon_hooks" in sys.modules:
        return
    mod = types.ModuleType("antenv.axon_hooks")
    mod._hook = None
    mod.set_axon_ntff_profile_hook = lambda h: setattr(mod, "_hook", h)
    mod.get_axon_ntff_profile_hook = lambda: mod._hook
    sys.modules["antenv.axon_hooks"] = mod
    sys.path.insert(0, "/root/.axon_site")
    from trn_agent_boot.trn_boot import _ntff_profile_via_ctypes

    mod.set_axon_ntff_profile_hook(
        _ntff_profile_via_ctypes("/opt/axon/libaxon_pjrt.so")
    )


class _Unit:
    """One (head-pair, q-chunk-of-512) attention work unit."""

    def __init__(self, pi, j, slot):
        self.pi = pi
        self.j = j
        self.slot = slot
        self.nk = 4 * (j + 1)
        self.kc = 0
        self.sq = deque()    # scores awaiting exp (1-step delay)
        self.pend = deque()  # exp'd items awaiting ctx (1-step delay)
        self.cur8 = None     # fp8 pt tile collecting the current kc pair
        self.ctx = None


def _build():
    nc = bacc.Bacc("TRN2", target_bir_lowering=False)

    # host pre-packed layouts: per-partition contiguous lines
    xT = nc.dram_tensor("xT", [128, NJ * NCI * 512], BF16, kind="ExternalInput")
    # wq/wk packed per head-pair: [128, pair, ci, 128] so the first unit's
    # weight slices can stream ahead of the rest
    wqT = nc.dram_tensor("wqT", [128, NPAIR * NCI * 128], BF16, kind="ExternalInput")
    wkT = nc.dram_tensor("wkT", [128, NPAIR * NCI * 128], BF16, kind="ExternalInput")
    wvT = nc.dram_tensor("wvT", [128, NCI * OC], BF16, kind="ExternalInput")
    # bq | bk | mask packed into one small early DMA
    smlT = nc.dram_tensor("smlT", [128, 2 * NPAIR + NT16], F32,
                          kind="ExternalInput")
    bv = nc.dram_tensor("bv", [OC], F32, kind="ExternalInput")
    outT = nc.dram_tensor("outT", [OC, T], BF16, kind="ExternalOutput")

    with tile.TileContext(nc) as tc:
        consts = tc.alloc_tile_pool(name="consts", bufs=1)
        qk_pool = tc.alloc_tile_pool(name="qk", bufs=1)
        va_pool = tc.alloc_tile_pool(name="va", bufs=1)

        # ---- constants ----
        sml = consts.tile([128, 2 * NPAIR + NT16], F32, tag="sml")
        bq_t = sml[:, 0:NPAIR]
        bk_t = sml[:, NPAIR:2 * NPAIR]
        mk_t = sml[:, 2 * NPAIR:]
        bvr = consts.tile([128, NH, D], F32, tag="bvr")


        # persistent activations
        qT = qk_pool.tile([128, NPAIR, T], BF16, tag="qT")
        kT = qk_pool.tile([128, NPAIR, T], BF16, tag="kT")
        va16 = va_pool.tile([128, NT16, NH, D + 1], BF16, tag="va16")
        # padded to 72B per head so the DoubleRow pair stride (NH*72) is
        # 16B-aligned as checkMatmultPerfMode requires
        va8 = va_pool.tile([128, NT16, NH, 72], F8, tag="va8")
        ones = consts.tile([128, 1], F32, tag="ones", name="ones")
        nc.vector.memset(ones, 1.0)

        pin_p = tc.alloc_tile_pool(name="pin", bufs=1)
        xt = pin_p.tile([128, NJ, NCI, 512], BF16, tag="xt")
        wq_t = pin_p.tile([128, NPAIR, NCI, 128], BF16, tag="wq")
        wk_t = pin_p.tile([128, NPAIR, NCI, 128], BF16, tag="wk")
        wv_t = pin_p.tile([128, NCI, OC], BF16, tag="wv")

        # batched input loads: x chunks on sync, weights on scalar/gpsimd;
        # small tensors issue after the big ones so descriptor generation
        # for the critical weights starts immediately.
        PW = NCI * 128
        nc.sync.dma_start(out=sml, in_=smlT[:, :])
        nc.sync.dma_start(out=xt[:, 0], in_=xT[:, 0:3072])
        nc.scalar.dma_start(out=wq_t[:, 0], in_=wqT[:, 0:PW])
        nc.scalar.dma_start(out=wk_t[:, 0], in_=wkT[:, 0:PW])
        nc.gpsimd.dma_start(out=wv_t, in_=wvT[:, :])
        nc.scalar.dma_start(out=wq_t[:, 1:3], in_=wqT[:, PW:3 * PW])
        nc.scalar.dma_start(out=wk_t[:, 1:3], in_=wkT[:, PW:3 * PW])
        for tj in (1, 2, 3):
            nc.sync.dma_start(
                out=xt[:, tj], in_=xT[:, 3072 * tj:3072 * (tj + 1)]
            )
        nc.gpsimd.dma_start(
            out=bvr,
            in_=bv[:].partition_broadcast(128).rearrange(
                "p (h d) -> p h d", h=NH
            ),
        )

        # warm-up operands for HAM filler matmuls (no DMA dependency)
        warm = consts.tile([128, 512], BF16, tag="warm", name="warm")
        nc.vector.memset(warm, 0.0)
        # all-ones row for the PE-based Z broadcast in emit_norm
        ones64 = consts.tile([1, 64], BF16, tag="o64", name="o64")
        nc.vector.memset(ones64, 1.0)
        # preload the ACT exp table while input DMAs stream
        wexp = consts.tile([128, 1], F32, tag="wexp", name="wexp")
        nc.scalar.activation(wexp, warm[:, 0:1],
                             mybir.ActivationFunctionType.Exp)

        pps = tc.alloc_tile_pool(name="pps", bufs=1, space="PSUM")
        sp = tc.alloc_tile_pool(name="sp", bufs=2, space="PSUM")
        cx = tc.alloc_tile_pool(name="cx", bufs=1, space="PSUM")
        pt16_pool = tc.alloc_tile_pool(name="pt16", bufs=6)
        pt8_pool = tc.alloc_tile_pool(name="pt8", bufs=6)
        npool = tc.alloc_tile_pool(name="np", bufs=3)

        # ---- projection work units (emitted lazily, interleaved with
        # attention so the PE stays dense while ACT chews on exps) ----
        def qk_chain(w_t, b_t, dst, pi, tj, psum_tag=None):
            def emit():
                if psum_tag:
                    ps = cx.tile([128, 512], F32, tag=psum_tag, name="ps")
                else:
                    ps = pps.tile([128, 512], F32, tag="ps", name="ps")
                for ci in range(NCI):
                    nc.tensor.matmul(
                        ps,
                        w_t[:, pi, ci, :],
                        xt[:, tj, ci, :],
                        start=(ci == 0),
                        stop=(ci == NCI - 1),
                    )
                nc.vector.tensor_scalar_add(
                    dst[:, pi, 512 * tj:512 * (tj + 1)], ps, b_t[:, pi:pi + 1]
                )
            return emit

        def v_chain(t16):
            def emit():
                ps = pps.tile([128, OC], F32, tag="ps", name="ps")
                tj, t0 = t16 // 4, 128 * (t16 % 4)
                for ci in range(NCI):
                    nc.tensor.matmul(
                        ps, xt[:, tj, ci, t0:t0 + 128], wv_t[:, ci, :],
                        start=(ci == 0), stop=(ci == NCI - 1),
                    )
                nc.vector.tensor_tensor(
                    va16[:, t16, :, 0:D],
                    ps.rearrange("p (h d) -> p h d", h=NH),
                    bvr,
                    op=mybir.AluOpType.add,
                )
                nc.vector.tensor_copy(va16[:, t16, :, D], ones.to_broadcast([128, NH]))
                nc.vector.tensor_copy(va8[:, t16, :, 0:D + 1], va16[:, t16])
            return emit

        chains = {}
        # j=0 units first (smallest data need), then long (ACT-heavy) j=3
        # units interleaved with short ones so exp work spreads out and the
        # projection chains (PE filler) last the whole kernel.
        unit_order = [(0, 0), (0, 1), (0, 2),
                      (1, 0), (3, 0), (1, 1), (2, 0), (3, 1),
                      (1, 2), (2, 1), (3, 2), (2, 2)]
        for pi in range(NPAIR):
            for tj in range(NJ):
                chains[f"q{pi}{tj}"] = qk_chain(wq_t, bq_t, qT, pi, tj)
                chains[f"k{pi}{tj}"] = qk_chain(wk_t, bk_t, kT, pi, tj)
        # the very first k chain accumulates in a ctx bank that's idle until
        # unit 1, so it doesn't WAR-serialize behind q00's bias-add
        chains["k00"] = qk_chain(wk_t, bk_t, kT, 0, 0, psum_tag="ca1")
        for t16 in range(NT16):
            chains[f"v{t16}"] = v_chain(t16)

        # chains first needed by each unit, in need order
        need = []
        _seen = set()
        for (j, pi) in unit_order:
            lst = []
            for n in ([f"q{pi}{j}"] + [f"k{pi}{t}" for t in range(j + 1)]
                      + [f"v{t}" for t in range(4 * (j + 1))]):
                if n not in _seen:
                    _seen.add(n)
                    lst.append(n)
            need.append(lst)
        # first unit: only q/k/v0 up front; v1-3 emit just-in-time at ctx
        need[0] = ["q00", "k00", "v0"]
        gall = deque(n for lst in need for n in lst)
        done = set()

        def emit_chain(name):
            if name not in done:
                done.add(name)
                chains[name]()

        def filler():
            wp = pps.tile([128, 512], F32, tag="ps", name="fil")
            nc.tensor.matmul(wp, warm[:, 0:128], warm,
                             start=True, stop=True)

        # HAM warm-up: keep the PE busy while input DMAs stream in
        for _ in range(7):
            wp = pps.tile([128, 512], F32, tag="ps", name="fil")
            nc.tensor.matmul(wp, warm[:, 0:128], warm,
                             start=True, stop=True)

        # ---- attention ----
        def emit_scores(u):
            kc = u.kc
            u.kc += 1
            c0 = max(0, kc - 4 * u.j) * 128
            emit_chain(f"k{u.pi}{kc // 4}")
            s2 = sp.tile([128, 2, 512], F32, tag="s", name="s2")
            for half in range(2):
                rows = slice(64 * half, 64 * half + 64)
                nc.tensor.matmul(
                    s2[:, half, c0:],
                    kT[rows, u.pi, 128 * kc:128 * (kc + 1)],
                    qT[rows, u.pi, 512 * u.j + c0:512 * (u.j + 1)],
                    start=True, stop=True,
                )
            u.sq.append((kc, c0, s2))

        def emit_exp(u):
            kc, c0, s2 = u.sq.popleft()
            if kc >= 4 * u.j:
                # diagonal chunk: bf16 P, triangular zeroing
                pt = pt16_pool.tile([128, 2, 512], BF16, tag="pt", name="pt")
                nc.scalar.activation(
                    pt[:, :, c0:], s2[:, :, c0:],
                    mybir.ActivationFunctionType.Exp,
                    bias=mk_t[:, kc:kc + 1], scale=0.125,
                )
                for half in range(2):
                    nc.gpsimd.affine_select(
                        out=pt[:, half, c0:c0 + 128],
                        in_=pt[:, half, c0:c0 + 128],
                        compare_op=mybir.AluOpType.is_ge,
                        fill=0.0,
                        base=0,
                        pattern=[[1, 128]],
                        channel_multiplier=-1,
                    )
                u.pend.append(("d", kc, c0, pt))
            else:
                # off-diagonal chunk: fp8 P into the kc-pair slot
                parity = kc % 2
                if parity == 0:
                    u.cur8 = pt8_pool.tile([128, 2, 2, 512], F8,
                                           tag="p8", name="p8")
                nc.scalar.activation(
                    u.cur8[:, parity], s2,
                    mybir.ActivationFunctionType.Exp,
                    bias=mk_t[:, kc:kc + 1], scale=0.125,
                )
                if parity == 1:
                    u.pend.append(("p8", kc - 1, 0, u.cur8))
                    u.cur8 = None

        def emit_step(u, step_i):
            emit_scores(u)
            if len(u.sq) > 1:
                emit_exp(u)
            # paced prefetch of the NEXT unit's projection chains
            if u.pref:
                tgt = (u.pref_n * u.kc + u.nk - 1) // u.nk
                while u.pref and u.pref_done < tgt:
                    emit_chain(u.pref.popleft())
                    u.pref_done += 1
            elif u.tail and step_i % 2 == 0:
                filler()
            if u.j == 0:
                # input-bound phase: PE has slack, pull chains forward
                while gall and gall[0] in done:
                    gall.popleft()
                if gall:
                    emit_chain(gall.popleft())
            if len(u.pend) > 1:
                emit_ctx(u, u.pend.popleft())

        def emit_ctx(u, item):
            kind, kc, c0, pt = item
            emit_chain(f"v{kc}")
            if kind == "p8":
                emit_chain(f"v{kc + 1}")
                for half in range(2):
                    nc.tensor.matmul(
                        u.ctx[half],
                        va8[:, kc:kc + 2, 2 * u.pi + half, 0:D + 1],
                        pt[:, :, half, :],
                        perf_mode=DR,
                        start=(kc == 0),
                        stop=False,
                    )
            else:
                for half in range(2):
                    nc.tensor.matmul(
                        u.ctx[half][:, c0:],
                        va16[:, kc, 2 * u.pi + half, :],
                        pt[:, half, c0:],
                        start=(kc == 0),
                        stop=(kc == u.nk - 1),
                    )

        def emit_norm(u):
            for half in range(2):
                hl = 2 * u.pi + half
                zr = npool.tile([1, 512], F32, tag="zr", name="zr")
                nc.vector.tensor_copy(zr, u.ctx[half][D:D + 1, :])
                zrec = npool.tile([1, 512], F32, tag="zrec", name="zrec")
                nc.vector.reciprocal_approx_fast(zrec, zr)
                zrep = npool.tile([64, 512], F32, tag="zrep", name="zrep")
                nc.gpsimd.partition_broadcast(zrep, zrec)
                ot = npool.tile([64, 512], BF16, tag="ot", name="ot")
                nc.vector.tensor_tensor(
                    ot, u.ctx[half][0:D, :], zrep, op=mybir.AluOpType.mult
                )
                nc.sync.dma_start(
                    out=outT[D * hl:D * (hl + 1), 512 * u.j:512 * (u.j + 1)],
                    in_=ot,
                )

        step_i = 0
        units = [_Unit(pi, j, 0) for (j, pi) in unit_order]
        for i, u in enumerate(units):
            u.pref = deque(need[i + 1]) if i + 1 < len(units) else deque()
            u.pref_n = len(u.pref)
            u.pref_done = 0
            u.tail = i >= len(units) - 2
            for nm in need[i]:
                emit_chain(nm)
            u.ctx = (
                cx.tile([D + 1, 512], F32, tag=f"ca{i % 2}", name="ctxa"),
                cx.tile([D + 1, 512], F32, tag="cb", name="ctxb"),
            )
            while u.kc < u.nk:
                emit_step(u, step_i)
                step_i += 1
            # hide this unit's drain under the next unit's pipeline head
            nxt = units[i + 1] if i + 1 < len(units) else None
            if nxt is not None:
                emit_chain(f"q{nxt.pi}{nxt.j}")
                emit_scores(nxt)
            while u.sq:
                emit_exp(u)
                if nxt is not None and nxt.kc < min(2, nxt.nk):
                    emit_scores(nxt)
            while u.pend:
                emit_ctx(u, u.pend.popleft())
            emit_norm(u)

        npool.release()
        pt8_pool.release()
        pt16_pool.release()
        cx.release()
        sp.release()
        pps.release()
        pin_p.release()
        va_pool.release()
        qk_pool.release()
        consts.release()

    nc.compile()
    return nc


def kernel(**inputs):
    global _COMPILED, LAST_EXEC_NS
    hs = np.asarray(inputs["hidden_states"], dtype=np.float32)
    am = np.asarray(inputs["attention_mask"], dtype=np.float32)
    Wq = np.asarray(inputs["Wq"], dtype=np.float32)
    bq = np.asarray(inputs["bq"], dtype=np.float32)
    Wk = np.asarray(inputs["Wk"], dtype=np.float32)
    bk = np.asarray(inputs["bk"], dtype=np.float32)
    Wv = np.asarray(inputs["Wv"], dtype=np.float32)
    bv = np.asarray(inputs["bv"], dtype=np.float32)

    if _COMPILED is None:
        _COMPILED = _build()
    nc = _COMPILED

    c = np.ascontiguousarray
    bf = ml_dtypes.bfloat16
    f8 = ml_dtypes.float8_e4m3

    def pack_x(xTb):
        # [768, 2048] -> [128, j, ci, 512] flattened, contiguous lines
        return c(xTb.reshape(NCI, 128, NJ, 512).transpose(1, 2, 0, 3)
                 .reshape(128, NJ * NCI * 512))

    def pack_w(WTb):
        # [768, 384] -> [128, ci, 384] flattened
        return c(WTb.reshape(NCI, 128, OC).transpose(1, 0, 2)
                 .reshape(128, NCI * OC))

    def pack_w_pair(WTb):
        # [768, 384] -> [128, pair, ci, 128] flattened
        return c(WTb.reshape(NCI, 128, NPAIR, 128).transpose(1, 2, 0, 3)
                 .reshape(128, NPAIR * NCI * 128))

    in_maps = []
    for core in range(8):
        b, half = core // 2, core % 2
        o0 = OC * half
        sl = slice(o0, o0 + OC)
        in_maps.append({
            "xT": pack_x(hs[b].T.astype(bf)),
            "wqT": pack_w_pair(Wq[sl, :].T.astype(bf)),
            "wkT": pack_w_pair(Wk[sl, :].T.astype(bf)),
            "wvT": pack_w(Wv[sl, :].T.astype(bf)),
            "smlT": c(np.concatenate([
                bq[sl].reshape(NPAIR, 128).T,
                bk[sl].reshape(NPAIR, 128).T,
                am[b, 0, 0, :].reshape(NT16, 128).T,
            ], axis=1)),
            "bv": c(bv[sl]),
        })

    if _TRACE:
        _install_trace_hook()
    res = run_bass_kernel_spmd(
        nc, in_maps, list(range(8)), trace=_TRACE, tmpdir=_TMPDIR
    )
    LAST_EXEC_NS = res.exec_time_ns

    out = np.empty((B, T, HID), dtype=np.float32)
    for core in range(8):
        b, half = core // 2, core % 2
        out[b, :, OC * half:OC * (half + 1)] = (
            res.results[core]["outT"].astype(np.float32).T
        )
    return out
